# revision 2
# baseline (speedup 1.0000x reference)
"""ALSH ConvNet Trainium2 kernel v2: mask-aware channel-sparse tiling.

Host computes the ALSH hashes (layer-1 query hash from x patch sums;
layer-2 query hash from a host conv1 — the ALSH query needs full-conv1
patch sums regardless), yielding per-image live channel sets:
  live1[b] = channels with fh1 == qh1[b]   (conv1 output channels)
  live2[b] = channels with fh2 == qh2[b]   (conv2 output channels)
Dead channels are provably zero downstream (mask commutes with relu and
the linear), so the device only computes live channels; masked output
rows are just bl (host fills).

Device program (SPMD, one core = IPC images x TPI row-tiles):
  conv1 banded matmul (K=3ci x (UW+2) x-rows, M=n1 x UW h1-rows, stream
  w) -> relu drain -> conv2 TRANSPOSED (stationary = h1 slab columns,
  moving = per-image banded W2 consts; out[w, (co,hr)]) -> relu drain
  -> fused transposed linear (stationary = h2T, moving = Wl chunks,
  N=10) -> out [n2*stride, 10] per tile. Host scatters + bl.
"""
import numpy as np
import concourse.bass as bass
from concourse import bacc
import concourse.tile as tile
import concourse.mybir as mybir
from concourse.bass_utils import run_bass_kernel_spmd

f32 = mybir.dt.float32
f16 = mybir.dt.float16
AF = mybir.ActivationFunctionType
ALU = mybir.AluOpType

R = 0.1
U = 0.99
N_CORES = 8
H, WX = 260, 1004
H1R, W1W = 258, 1002   # conv1 out rows/cols
H2R, W2W = 256, 1000   # conv2 out rows/cols
W1CH = [(0, 512), (512, 490)]          # conv1 psum column chunks
LKS = [(k * 128, 128) for k in range(7)] + [(896, 104)]  # w chunks


def _filter_hash(W, a, b):
    Cout = W.shape[0]
    wf = W.reshape(Cout, -1).astype(np.float32)
    norms = np.sqrt((wf * wf).sum(1))
    ws = wf * np.float32(U / norms.max())
    n2 = (ws * ws).sum(1)
    powers = np.stack([n2, n2**2, n2**4, n2**8, n2**16], axis=1)
    Pw = np.concatenate([ws, powers], axis=1).astype(np.float32)
    return np.mod(np.floor(
        (Pw @ a.astype(np.float32) + np.float32(b)) / np.float32(R)
    ).astype(np.int64), 2).astype(np.int64)


def _qhash(q, a, b):
    qn = q / np.maximum(np.linalg.norm(q, axis=1, keepdims=True), 1e-12)
    v = qn @ a[:q.shape[1]].astype(np.float64) \
        + 0.5 * a[q.shape[1]:].astype(np.float64).sum() + float(b)
    return np.mod(np.floor(v / R).astype(np.int64), 2)


def _build_l1(W1g, n1s, UW):
    # [3ci*(UW+2), 3dj * (n1s*UW)]; col (dj, co, u), row (ci, dh):
    # value = W1g[co, ci, dh-u, dj] for 0 <= dh-u < 3
    KH = UW + 2
    M = n1s * UW
    L = np.zeros((3 * KH, 3 * M), np.float32)
    n1 = W1g.shape[0]
    for dj in range(3):
        for co in range(n1):
            for ci in range(3):
                for u in range(UW):
                    for di in range(3):
                        L[ci * KH + u + di, dj * M + co * UW + u] = \
                            W1g[co, ci, di, dj]
    return L


def _build_w2(W2g, n1s, n2s, UW, ST):
    # [n1s*UW, 3dw * (n2s*ST)]; row (ci, u), col (dw, co, hr):
    # value = W2g[co, ci, u-hr, dw] for 0 <= u-hr < 3
    F = n2s * ST
    Bm = np.zeros((n1s * UW, 3 * F), np.float32)
    n2, n1 = W2g.shape[0], W2g.shape[1]
    for dw in range(3):
        for co in range(n2):
            for ci in range(n1):
                for hr in range(ST):
                    for dh in range(3):
                        u = hr + dh
                        if u < UW:
                            Bm[ci * UW + u, dw * F + co * ST + hr] = \
                                W2g[co, ci, dh, dw]
    return Bm


def _build_wlt(Wln):
    WlT = Wln.T.astype(np.float32)             # [1000, 10]
    wltf = np.zeros((128, 80), np.float32)
    for k, (k0, K) in enumerate(LKS):
        wltf[:K, k * 10:k * 10 + 10] = WlT[k0:k0 + K]
    return wltf


def _build_nc(TPI, IPC, UW, n1s, n2s):
    """TPI tiles/image, IPC images/core, UW-row h1 window (stride UW-2),
    n1s/n2s live-channel slots."""
    ST = UW - 2
    KH = UW + 2
    K1 = 3 * KH                 # conv1 contraction rows (x slab)
    M1 = n1s * UW               # conv1 out partitions
    K2 = n1s * UW               # conv2 contraction rows (h1 slab)
    F2 = n2s * ST               # conv2 free (co, hr)
    CPB = max(1, 512 // F2)     # conv2 chunks per psum bank
    NB = (len(LKS) + CPB - 1) // CPB   # banks per tile
    TPC = TPI * IPC

    nc = bacc.Bacc("TRN2", target_bir_lowering=False)
    xsP = nc.declare_dram_parameter("xs", [TPC, K1, WX], f16, isOutput=False)
    l1P = nc.declare_dram_parameter("l1", [IPC, K1, 3 * M1], f16,
                                    isOutput=False)
    w2P = nc.declare_dram_parameter("w2", [IPC, K2, 3 * F2], f16,
                                    isOutput=False)
    wltP = nc.declare_dram_parameter("wlt", [128, 80], f16, isOutput=False)
    outP = nc.declare_dram_parameter("out", [IPC, F2, TPI * 10], f32,
                                     isOutput=True)
    STL = H2R - ST * (TPI - 1)      # valid out rows in the tail tile

    with tile.TileContext(nc) as tc:
        with tc.tile_pool(name="consts", bufs=1) as cpool, \
             tc.tile_pool(name="imc", bufs=3) as imc, \
             tc.tile_pool(name="xcp", bufs=5) as xcp, \
             tc.tile_pool(name="h1p", bufs=4) as h1p, \
             tc.tile_pool(name="h2p", bufs=4) as h2p, \
             tc.tile_pool(name="outp", bufs=2) as outp, \
             tc.tile_pool(name="c1ps", bufs=4, space="PSUM") as c1ps, \
             tc.tile_pool(name="c2ps", bufs=2, space="PSUM") as c2ps, \
             tc.tile_pool(name="lps", bufs=2, space="PSUM") as lps:

            l1s, w2s = {}, {}
            xcs, h1s, h2s = {}, {}, {}

            def load_x(ti):
                xc = xcp.tile([K1, WX], f16, tag="xc")
                nc.sync.dma_start(xc[:], xsP[ti])
                xcs[ti] = xc

            def load_img_consts(im, split=False):
                t1 = imc.tile([K1, 3 * M1], f16, tag="l1d")
                nc.sync.dma_start(t1[:], l1P[im])
                l1s[im] = t1
                if split:
                    load_x(0)
                    load_x(1)
                t2 = imc.tile([K2, 3 * F2], f16, tag="w2d")
                nc.sync.dma_start(t2[:], w2P[im])
                w2s[im] = t2

            # PE warm-up during the initial DMA latency: zeros via gpsimd
            # memset, then throwaway matmuls so the p-state ramp completes
            # before real work arrives. Uses an lps psum generation that is
            # recycled long before the first real linear group.
            wz = cpool.tile([128, 16], f16, tag="wz")
            nc.gpsimd.memset(wz[:], 0.0)
            wps = lps.tile([8, 512], f32, tag="lps")
            for _ in range(1):
                nc.tensor.matmul(wps[0:8, 0:16], wz[:, 0:8], wz[:, 0:16],
                                 start=True, stop=True, skip_group_check=True)

            load_img_consts(0, split=True)

            # wlt: DMA then DVE copy (engine-produced matmul operand);
            # first needed by linear at ti=2, so loaded after the hot path.
            wlts = cpool.tile([128, 80], f16, tag="wlt_d")
            nc.sync.dma_start(wlts[:], wltP[:])

            def conv1(ti):
                im = ti // TPI
                h1 = h1p.tile([M1, W1W], f16, tag="h1")
                xc = xcs.pop(ti)
                for wi, (w0, N) in enumerate(W1CH):
                    ps = c1ps.tile([M1, 512], f32, tag="c1ps")
                    for dj in range(3):
                        if isinstance(xc, tuple):
                            src = xc[wi][:, dj:dj + N] if wi == 0                                 else xc[1][:, dj:dj + N]
                        else:
                            src = xc[:, w0 + dj:w0 + dj + N]
                        nc.tensor.matmul(
                            ps[0:M1, 0:N],
                            l1s[im][:, dj * M1:(dj + 1) * M1],
                            src,
                            start=(dj == 0), stop=(dj == 2))
                    if wi == 0:
                        nc.scalar.activation(h1[:, w0:w0 + N], ps[0:M1, 0:N],
                                             AF.Relu)
                    else:
                        nc.vector.tensor_scalar_max(h1[:, w0:w0 + N],
                                                    ps[0:M1, 0:N], 0.0)
                h1s[ti] = h1

            def conv2(ti):
                im, t = divmod(ti, TPI)
                tail = (t == TPI - 1)
                STt = STL if tail else ST
                h1 = h1s.pop(ti)
                h2 = h2p.tile([128, len(LKS) * F2], f16, tag="h2")
                for b in range(NB):
                    ks = range(b * CPB, min((b + 1) * CPB, len(LKS)))
                    ps = c2ps.tile([128, 512], f32, tag="c2ps")
                    for ci_, k in enumerate(ks):
                        w0, M = LKS[k]
                        for dw in range(3):
                            rhs = w2s[im][:, dw * F2:(dw + 1) * F2]
                            dst = ps[0:M, ci_ * F2:ci_ * F2 + F2]
                            if tail:
                                rhs = rhs.rearrange(
                                    "p (c h) -> p c h", c=n2s)[:, :, 0:STt]
                                dst = dst.rearrange(
                                    "p (c h) -> p c h", c=n2s)[:, :, 0:STt]
                            nc.tensor.matmul(
                                dst, h1[:, w0 + dw:w0 + dw + M], rhs,
                                start=(ci_ == 0 and dw == 0),
                                stop=(ci_ == len(ks) - 1 and dw == 2),
                                skip_group_check=True)
                    ncols = len(ks) * F2
                    dst = h2[:, b * CPB * F2: b * CPB * F2 + ncols]
                    if b % 2 == 0:
                        nc.scalar.activation(dst, ps[0:128, 0:ncols], AF.Relu)
                    else:
                        nc.vector.tensor_scalar_max(dst, ps[0:128, 0:ncols],
                                                    0.0)
                h2s[ti] = h2

            pls, obs = {}, {}

            def linear(ti):
                im, t = divmod(ti, TPI)
                h2 = h2s.pop(ti)
                if t == 0:
                    pls[im] = lps.tile([F2, TPI * 10], f32, name="plin",
                                       tag="lps")
                pl = pls[im]
                for k, (k0, K) in enumerate(LKS):
                    nc.tensor.matmul(pl[0:F2, t * 10:t * 10 + 10],
                                     h2[0:K, k * F2:k * F2 + F2],
                                     wlts[0:K, k * 10:k * 10 + 10],
                                     start=(t == 0 and k == 0),
                                     stop=(t == TPI - 1 and k == len(LKS) - 1),
                                     skip_group_check=True)
                if t == TPI - 1:
                    pl = pls.pop(im)
                    ob = outp.tile([F2, TPI * 10], f32, tag="outsb")
                    nc.vector.tensor_copy(ob[:], pl[0:F2, :])
                    nc.sync.dma_start(outP[im], ob[:])

            # software pipeline: conv1(t) | conv2(t-1) | linear(t-2)
            for ti in range(TPC + 2):
                if ti < TPC:
                    if ti % TPI == 0 and ti // TPI + 1 < IPC:
                        load_img_consts(ti // TPI + 1)
                    conv1(ti)
                    if ti + 2 < TPC:
                        load_x(ti + 2)
                if 1 <= ti <= TPC:
                    conv2(ti - 1)
                if 2 <= ti:
                    linear(ti - 2)
    nc.compile()
    return nc


_CACHE = {}
LAST_RES = None


def _host_conv1(x, W1g):
    # relu(conv1) for gathered live channels; x [B,3,260,1004] f32,
    # W1g [B, n1s, 3, 3, 3] per-image gathered weights (zero-padded).
    from numpy.lib.stride_tricks import sliding_window_view
    win = sliding_window_view(x, (3, 3), axis=(2, 3))  # [B,3,258,1002,3,3]
    h = np.einsum("bchwij,bkcij->bkhw", win, W1g, optimize=True)
    return np.maximum(h, 0.0, out=h)


def kernel(x, W1, b1, W2, a1, a2, b2, Wl, bl, **kw):
    x = np.asarray(x, np.float32)
    W1n = np.asarray(W1, np.float32)
    W2n = np.asarray(W2, np.float32)
    a1n = np.asarray(a1, np.float32)
    a2n = np.asarray(a2, np.float32)
    b1n = float(np.asarray(b1, np.float32))
    b2n = float(np.asarray(b2, np.float32))
    Wln = np.asarray(Wl, np.float32)
    bln = np.asarray(bl, np.float32)
    B = x.shape[0]

    # ---- host: ALSH hashes -> per-image live channel sets ----
    fh1 = _filter_hash(W1n, a1n, b1n)
    fh2 = _filter_hash(W2n, a2n, b2n)
    q1v = np.empty((B, 27), np.float64)
    for i in range(3):
        for j in range(3):
            s = x[:, :, i:i + H1R, j:j + W1W].sum(axis=(2, 3),
                                                  dtype=np.float64)
            for ci in range(3):
                q1v[:, ci * 9 + i * 3 + j] = s[:, ci]
    qh1 = _qhash(q1v, a1n, b1n)
    mask1 = (fh1[None, :] == qh1[:, None])              # [B, 5] bool
    live1 = [np.where(mask1[b])[0] for b in range(B)]
    n1s = max(1, max(len(v) for v in live1))

    # gathered conv1 weights (zero-padded to n1s slots)
    W1g = np.zeros((B, n1s, 3, 3, 3), np.float32)
    for b in range(B):
        for k, c in enumerate(live1[b]):
            W1g[b, k] = W1n[c]

    # layer-2 query hash needs full-conv1 patch sums -> host conv1
    h1h = _host_conv1(x, W1g)                           # [B, n1s, 258, 1002]
    q2v = np.zeros((B, 45), np.float64)
    for i in range(3):
        for j in range(3):
            s = h1h[:, :, i:i + H2R, j:j + W2W].sum(axis=(2, 3),
                                                    dtype=np.float64)
            for b in range(B):
                for k, c in enumerate(live1[b]):
                    q2v[b, c * 9 + i * 3 + j] = s[b, k]
    qh2 = _qhash(q2v, a2n, b2n)
    mask2 = (fh2[None, :] == qh2[:, None])              # [B, 5] bool
    live2 = [np.where(mask2[b])[0] if len(live1[b]) else np.empty(0, np.int64)
             for b in range(B)]
    n2s = max(1, max((len(v) for v in live2), default=1))

    out_full = np.broadcast_to(
        bln[None, None, None, :], (B, 5, H2R, 10)).astype(np.float32).copy()

    work = [b for b in range(B) if len(live1[b]) and len(live2[b])]
    if not work:
        return out_full

    # ---- geometry ----
    UW = min(40, 128 // n1s)       # h1 slab rows (K2 = n1s*UW <= 128 etc.)
    ST = UW - 2
    TPI = -(-H2R // ST)            # tiles per image
    IPC = -(-len(work) // N_CORES)  # images per core (padded)
    K1 = 3 * (UW + 2)
    M1 = n1s * UW
    F2 = n2s * ST

    key = (TPI, IPC, UW, n1s, n2s)
    if key not in _CACHE:
        _CACHE.clear()
        _CACHE[key] = _build_nc(TPI, IPC, UW, n1s, n2s)
    nc = _CACHE[key]

    # ---- host prep: per-core inputs ----
    x16 = x.astype(np.float16)
    wltf = _build_wlt(Wln).astype(np.float16)
    TPC = TPI * IPC
    in_maps = []
    core_imgs = []
    for c in range(N_CORES):
        imgs = work[c * IPC:(c + 1) * IPC]
        core_imgs.append(imgs)
        xs = np.zeros((TPC, K1, WX), np.float16)
        l1c = np.zeros((IPC, K1, 3 * M1), np.float16)
        w2c = np.zeros((IPC, K2loc := n1s * UW, 3 * F2), np.float16)
        for il, b in enumerate(imgs):
            W2g = np.zeros((n2s, n1s, 3, 3), np.float32)
            for ko, co in enumerate(live2[b]):
                for ki, ci in enumerate(live1[b]):
                    W2g[ko, ki] = W2n[co, ci]
            l1c[il] = _build_l1(W1g[b], n1s, UW).astype(np.float16)
            w2c[il] = _build_w2(W2g, n1s, n2s, UW, ST).astype(np.float16)
            for t in range(TPI):
                r0 = ST * t
                nrow = min(UW + 2, H - r0)
                sl = x16[b, :, r0:r0 + nrow, :]           # [3, nrow, 1004]
                dst = xs[il * TPI + t].reshape(3, UW + 2, WX)
                dst[:, :nrow, :] = sl
        in_maps.append({"xs": xs, "l1": l1c, "w2": w2c, "wlt": wltf})

    res = run_bass_kernel_spmd(nc, in_maps, core_ids=list(range(N_CORES)),
                               **kw)
    global LAST_RES
    LAST_RES = res

    # ---- host: scatter ----
    for c in range(N_CORES):
        lin = res.results[c]["out"]                     # [IPC, F2, TPI*10]
        for il, b in enumerate(core_imgs[c]):
            for t in range(TPI):
                r0 = ST * t
                nr = min(ST, H2R - r0)
                blk = lin[il][:, t * 10:(t + 1) * 10].reshape(n2s, ST, 10)
                for ko, co in enumerate(live2[b]):
                    out_full[b, co, r0:r0 + nr, :] = blk[ko, :nr, :] \
                        + bln[None, :]
    return out_full


# revision 3
# speedup vs baseline: 1.0024x; 1.0024x over previous
"""ALSH ConvNet Trainium2 kernel v2: mask-aware channel-sparse tiling.

Host computes the ALSH hashes (layer-1 query hash from x patch sums;
layer-2 query hash from a host conv1 — the ALSH query needs full-conv1
patch sums regardless), yielding per-image live channel sets:
  live1[b] = channels with fh1 == qh1[b]   (conv1 output channels)
  live2[b] = channels with fh2 == qh2[b]   (conv2 output channels)
Dead channels are provably zero downstream (mask commutes with relu and
the linear), so the device only computes live channels; masked output
rows are just bl (host fills).

Device program (SPMD, one core = IPC images x TPI row-tiles):
  conv1 banded matmul (K=3ci x (UW+2) x-rows, M=n1 x UW h1-rows, stream
  w) -> relu drain -> conv2 TRANSPOSED (stationary = h1 slab columns,
  moving = per-image banded W2 consts; out[w, (co,hr)]) -> relu drain
  -> fused transposed linear (stationary = h2T, moving = Wl chunks,
  N=10) -> out [n2*stride, 10] per tile. Host scatters + bl.
"""
import numpy as np
import concourse.bass as bass
from concourse import bacc
import concourse.tile as tile
import concourse.mybir as mybir
from concourse.bass_utils import run_bass_kernel_spmd

f32 = mybir.dt.float32
f16 = mybir.dt.float16
AF = mybir.ActivationFunctionType
ALU = mybir.AluOpType

R = 0.1
U = 0.99
N_CORES = 8
H, WX = 260, 1004
H1R, W1W = 258, 1002   # conv1 out rows/cols
H2R, W2W = 256, 1000   # conv2 out rows/cols
W1CH = [(0, 512), (512, 490)]          # conv1 psum column chunks
LKS = [(k * 128, 128) for k in range(7)] + [(896, 104)]  # w chunks


def _filter_hash(W, a, b):
    Cout = W.shape[0]
    wf = W.reshape(Cout, -1).astype(np.float32)
    norms = np.sqrt((wf * wf).sum(1))
    ws = wf * np.float32(U / norms.max())
    n2 = (ws * ws).sum(1)
    powers = np.stack([n2, n2**2, n2**4, n2**8, n2**16], axis=1)
    Pw = np.concatenate([ws, powers], axis=1).astype(np.float32)
    return np.mod(np.floor(
        (Pw @ a.astype(np.float32) + np.float32(b)) / np.float32(R)
    ).astype(np.int64), 2).astype(np.int64)


def _qhash(q, a, b):
    qn = q / np.maximum(np.linalg.norm(q, axis=1, keepdims=True), 1e-12)
    v = qn @ a[:q.shape[1]].astype(np.float64) \
        + 0.5 * a[q.shape[1]:].astype(np.float64).sum() + float(b)
    return np.mod(np.floor(v / R).astype(np.int64), 2)


def _build_l1(W1g, n1s, UW):
    # [3ci*(UW+2), 3dj * (n1s*UW)]; col (dj, co, u), row (ci, dh):
    # value = W1g[co, ci, dh-u, dj] for 0 <= dh-u < 3
    KH = UW + 2
    M = n1s * UW
    L = np.zeros((3 * KH, 3 * M), np.float32)
    n1 = W1g.shape[0]
    for dj in range(3):
        for co in range(n1):
            for ci in range(3):
                for u in range(UW):
                    for di in range(3):
                        L[ci * KH + u + di, dj * M + co * UW + u] = \
                            W1g[co, ci, di, dj]
    return L


def _build_w2(W2g, n1s, n2s, UW, ST):
    # [n1s*UW, 3dw * (n2s*ST)]; row (ci, u), col (dw, co, hr):
    # value = W2g[co, ci, u-hr, dw] for 0 <= u-hr < 3
    F = n2s * ST
    Bm = np.zeros((n1s * UW, 3 * F), np.float32)
    n2, n1 = W2g.shape[0], W2g.shape[1]
    for dw in range(3):
        for co in range(n2):
            for ci in range(n1):
                for hr in range(ST):
                    for dh in range(3):
                        u = hr + dh
                        if u < UW:
                            Bm[ci * UW + u, dw * F + co * ST + hr] = \
                                W2g[co, ci, dh, dw]
    return Bm


def _build_wlt(Wln):
    WlT = Wln.T.astype(np.float32)             # [1000, 10]
    wltf = np.zeros((128, 80), np.float32)
    for k, (k0, K) in enumerate(LKS):
        wltf[:K, k * 10:k * 10 + 10] = WlT[k0:k0 + K]
    return wltf


def _build_nc(TPI, IPC, UW, n1s, n2s):
    """TPI tiles/image, IPC images/core, UW-row h1 window (stride UW-2),
    n1s/n2s live-channel slots."""
    ST = UW - 2
    KH = UW + 2
    K1 = 3 * KH                 # conv1 contraction rows (x slab)
    M1 = n1s * UW               # conv1 out partitions
    K2 = n1s * UW               # conv2 contraction rows (h1 slab)
    F2 = n2s * ST               # conv2 free (co, hr)
    CPB = max(1, 512 // F2)     # conv2 chunks per psum bank
    NB = (len(LKS) + CPB - 1) // CPB   # banks per tile
    TPC = TPI * IPC

    nc = bacc.Bacc("TRN2", target_bir_lowering=False)
    xsP = nc.declare_dram_parameter("xs", [TPC, K1, WX], f16, isOutput=False)
    l1P = nc.declare_dram_parameter("l1", [IPC, K1, 3 * M1], f16,
                                    isOutput=False)
    w2P = nc.declare_dram_parameter("w2", [IPC, K2, 3 * F2], f16,
                                    isOutput=False)
    wltP = nc.declare_dram_parameter("wlt", [128, 80], f16, isOutput=False)
    outP = nc.declare_dram_parameter("out", [IPC, F2, TPI * 10], f32,
                                     isOutput=True)
    STL = H2R - ST * (TPI - 1)      # valid out rows in the tail tile

    with tile.TileContext(nc) as tc:
        with tc.tile_pool(name="consts", bufs=1) as cpool, \
             tc.tile_pool(name="imc", bufs=3) as imc, \
             tc.tile_pool(name="xcp", bufs=5) as xcp, \
             tc.tile_pool(name="h1p", bufs=4) as h1p, \
             tc.tile_pool(name="h2p", bufs=4) as h2p, \
             tc.tile_pool(name="outp", bufs=2) as outp, \
             tc.tile_pool(name="c1ps", bufs=4, space="PSUM") as c1ps, \
             tc.tile_pool(name="c2ps", bufs=2, space="PSUM") as c2ps, \
             tc.tile_pool(name="lps", bufs=2, space="PSUM") as lps:

            l1s, w2s = {}, {}
            xcs, h1s, h2s = {}, {}, {}

            def load_x(ti):
                xc = xcp.tile([K1, WX], f16, tag="xc")
                nc.sync.dma_start(xc[:], xsP[ti])
                xcs[ti] = xc

            def load_img_consts(im, split=False):
                if split:
                    load_x(0)
                t1 = imc.tile([K1, 3 * M1], f16, tag="l1d")
                nc.sync.dma_start(t1[:], l1P[im])
                l1s[im] = t1
                if split:
                    load_x(1)
                t2 = imc.tile([K2, 3 * F2], f16, tag="w2d")
                nc.sync.dma_start(t2[:], w2P[im])
                w2s[im] = t2

            # PE warm-up during the initial DMA latency: zeros via gpsimd
            # memset, then throwaway matmuls so the p-state ramp completes
            # before real work arrives. Uses an lps psum generation that is
            # recycled long before the first real linear group.
            wz = cpool.tile([128, 16], f16, tag="wz")
            nc.gpsimd.memset(wz[:], 0.0)
            wps = lps.tile([8, 512], f32, tag="lps")
            for _ in range(1):
                nc.tensor.matmul(wps[0:8, 0:16], wz[:, 0:8], wz[:, 0:16],
                                 start=True, stop=True, skip_group_check=True)

            load_img_consts(0, split=True)

            # wlt: DMA then DVE copy (engine-produced matmul operand);
            # first needed by linear at ti=2, so loaded after the hot path.
            wlts = cpool.tile([128, 80], f16, tag="wlt_d")
            nc.sync.dma_start(wlts[:], wltP[:])

            def conv1(ti):
                im = ti // TPI
                h1 = h1p.tile([M1, W1W], f16, tag="h1")
                xc = xcs.pop(ti)
                for wi, (w0, N) in enumerate(W1CH):
                    ps = c1ps.tile([M1, 512], f32, tag="c1ps")
                    for dj in range(3):
                        if isinstance(xc, tuple):
                            src = xc[wi][:, dj:dj + N] if wi == 0                                 else xc[1][:, dj:dj + N]
                        else:
                            src = xc[:, w0 + dj:w0 + dj + N]
                        nc.tensor.matmul(
                            ps[0:M1, 0:N],
                            l1s[im][:, dj * M1:(dj + 1) * M1],
                            src,
                            start=(dj == 0), stop=(dj == 2))
                    if wi == 0:
                        nc.scalar.activation(h1[:, w0:w0 + N], ps[0:M1, 0:N],
                                             AF.Relu)
                    else:
                        nc.vector.tensor_scalar_max(h1[:, w0:w0 + N],
                                                    ps[0:M1, 0:N], 0.0)
                h1s[ti] = h1

            def conv2(ti):
                im, t = divmod(ti, TPI)
                tail = (t == TPI - 1)
                STt = STL if tail else ST
                h1 = h1s.pop(ti)
                h2 = h2p.tile([128, len(LKS) * F2], f16, tag="h2")
                for b in range(NB):
                    ks = range(b * CPB, min((b + 1) * CPB, len(LKS)))
                    ps = c2ps.tile([128, 512], f32, tag="c2ps")
                    for ci_, k in enumerate(ks):
                        w0, M = LKS[k]
                        for dw in range(3):
                            rhs = w2s[im][:, dw * F2:(dw + 1) * F2]
                            dst = ps[0:M, ci_ * F2:ci_ * F2 + F2]
                            if tail:
                                rhs = rhs.rearrange(
                                    "p (c h) -> p c h", c=n2s)[:, :, 0:STt]
                                dst = dst.rearrange(
                                    "p (c h) -> p c h", c=n2s)[:, :, 0:STt]
                            nc.tensor.matmul(
                                dst, h1[:, w0 + dw:w0 + dw + M], rhs,
                                start=(ci_ == 0 and dw == 0),
                                stop=(ci_ == len(ks) - 1 and dw == 2),
                                skip_group_check=True)
                    ncols = len(ks) * F2
                    dst = h2[:, b * CPB * F2: b * CPB * F2 + ncols]
                    if b % 2 == 0:
                        nc.scalar.activation(dst, ps[0:128, 0:ncols], AF.Relu)
                    else:
                        nc.vector.tensor_scalar_max(dst, ps[0:128, 0:ncols],
                                                    0.0)
                h2s[ti] = h2

            pls, obs = {}, {}

            def linear(ti):
                im, t = divmod(ti, TPI)
                h2 = h2s.pop(ti)
                if t == 0:
                    pls[im] = lps.tile([F2, TPI * 10], f32, name="plin",
                                       tag="lps")
                pl = pls[im]
                for k, (k0, K) in enumerate(LKS):
                    nc.tensor.matmul(pl[0:F2, t * 10:t * 10 + 10],
                                     h2[0:K, k * F2:k * F2 + F2],
                                     wlts[0:K, k * 10:k * 10 + 10],
                                     start=(t == 0 and k == 0),
                                     stop=(t == TPI - 1 and k == len(LKS) - 1),
                                     skip_group_check=True)
                if t == TPI - 1:
                    pl = pls.pop(im)
                    ob = outp.tile([F2, TPI * 10], f32, tag="outsb")
                    nc.vector.tensor_copy(ob[:], pl[0:F2, :])
                    nc.sync.dma_start(outP[im], ob[:])

            # software pipeline: conv1(t) | conv2(t-1) | linear(t-2)
            for ti in range(TPC + 2):
                if ti < TPC:
                    if ti % TPI == 0 and ti // TPI + 1 < IPC:
                        load_img_consts(ti // TPI + 1)
                    conv1(ti)
                    if ti + 2 < TPC:
                        load_x(ti + 2)
                if 1 <= ti <= TPC:
                    conv2(ti - 1)
                if 2 <= ti:
                    linear(ti - 2)
    nc.compile()
    return nc


_CACHE = {}
LAST_RES = None


def _host_conv1(x, W1g):
    # relu(conv1) for gathered live channels; x [B,3,260,1004] f32,
    # W1g [B, n1s, 3, 3, 3] per-image gathered weights (zero-padded).
    from numpy.lib.stride_tricks import sliding_window_view
    win = sliding_window_view(x, (3, 3), axis=(2, 3))  # [B,3,258,1002,3,3]
    h = np.einsum("bchwij,bkcij->bkhw", win, W1g, optimize=True)
    return np.maximum(h, 0.0, out=h)


def kernel(x, W1, b1, W2, a1, a2, b2, Wl, bl, **kw):
    x = np.asarray(x, np.float32)
    W1n = np.asarray(W1, np.float32)
    W2n = np.asarray(W2, np.float32)
    a1n = np.asarray(a1, np.float32)
    a2n = np.asarray(a2, np.float32)
    b1n = float(np.asarray(b1, np.float32))
    b2n = float(np.asarray(b2, np.float32))
    Wln = np.asarray(Wl, np.float32)
    bln = np.asarray(bl, np.float32)
    B = x.shape[0]

    # ---- host: ALSH hashes -> per-image live channel sets ----
    fh1 = _filter_hash(W1n, a1n, b1n)
    fh2 = _filter_hash(W2n, a2n, b2n)
    q1v = np.empty((B, 27), np.float64)
    for i in range(3):
        for j in range(3):
            s = x[:, :, i:i + H1R, j:j + W1W].sum(axis=(2, 3),
                                                  dtype=np.float64)
            for ci in range(3):
                q1v[:, ci * 9 + i * 3 + j] = s[:, ci]
    qh1 = _qhash(q1v, a1n, b1n)
    mask1 = (fh1[None, :] == qh1[:, None])              # [B, 5] bool
    live1 = [np.where(mask1[b])[0] for b in range(B)]
    n1s = max(1, max(len(v) for v in live1))

    # gathered conv1 weights (zero-padded to n1s slots)
    W1g = np.zeros((B, n1s, 3, 3, 3), np.float32)
    for b in range(B):
        for k, c in enumerate(live1[b]):
            W1g[b, k] = W1n[c]

    # layer-2 query hash needs full-conv1 patch sums -> host conv1
    h1h = _host_conv1(x, W1g)                           # [B, n1s, 258, 1002]
    q2v = np.zeros((B, 45), np.float64)
    for i in range(3):
        for j in range(3):
            s = h1h[:, :, i:i + H2R, j:j + W2W].sum(axis=(2, 3),
                                                    dtype=np.float64)
            for b in range(B):
                for k, c in enumerate(live1[b]):
                    q2v[b, c * 9 + i * 3 + j] = s[b, k]
    qh2 = _qhash(q2v, a2n, b2n)
    mask2 = (fh2[None, :] == qh2[:, None])              # [B, 5] bool
    live2 = [np.where(mask2[b])[0] if len(live1[b]) else np.empty(0, np.int64)
             for b in range(B)]
    n2s = max(1, max((len(v) for v in live2), default=1))

    out_full = np.broadcast_to(
        bln[None, None, None, :], (B, 5, H2R, 10)).astype(np.float32).copy()

    work = [b for b in range(B) if len(live1[b]) and len(live2[b])]
    if not work:
        return out_full

    # ---- geometry ----
    UW = min(40, 128 // n1s)       # h1 slab rows (K2 = n1s*UW <= 128 etc.)
    ST = UW - 2
    TPI = -(-H2R // ST)            # tiles per image
    IPC = -(-len(work) // N_CORES)  # images per core (padded)
    K1 = 3 * (UW + 2)
    M1 = n1s * UW
    F2 = n2s * ST

    key = (TPI, IPC, UW, n1s, n2s)
    if key not in _CACHE:
        _CACHE.clear()
        _CACHE[key] = _build_nc(TPI, IPC, UW, n1s, n2s)
    nc = _CACHE[key]

    # ---- host prep: per-core inputs ----
    x16 = x.astype(np.float16)
    wltf = _build_wlt(Wln).astype(np.float16)
    TPC = TPI * IPC
    in_maps = []
    core_imgs = []
    for c in range(N_CORES):
        imgs = work[c * IPC:(c + 1) * IPC]
        core_imgs.append(imgs)
        xs = np.zeros((TPC, K1, WX), np.float16)
        l1c = np.zeros((IPC, K1, 3 * M1), np.float16)
        w2c = np.zeros((IPC, K2loc := n1s * UW, 3 * F2), np.float16)
        for il, b in enumerate(imgs):
            W2g = np.zeros((n2s, n1s, 3, 3), np.float32)
            for ko, co in enumerate(live2[b]):
                for ki, ci in enumerate(live1[b]):
                    W2g[ko, ki] = W2n[co, ci]
            l1c[il] = _build_l1(W1g[b], n1s, UW).astype(np.float16)
            w2c[il] = _build_w2(W2g, n1s, n2s, UW, ST).astype(np.float16)
            for t in range(TPI):
                r0 = ST * t
                nrow = min(UW + 2, H - r0)
                sl = x16[b, :, r0:r0 + nrow, :]           # [3, nrow, 1004]
                dst = xs[il * TPI + t].reshape(3, UW + 2, WX)
                dst[:, :nrow, :] = sl
        in_maps.append({"xs": xs, "l1": l1c, "w2": w2c, "wlt": wltf})

    res = run_bass_kernel_spmd(nc, in_maps, core_ids=list(range(N_CORES)),
                               **kw)
    global LAST_RES
    LAST_RES = res

    # ---- host: scatter ----
    for c in range(N_CORES):
        lin = res.results[c]["out"]                     # [IPC, F2, TPI*10]
        for il, b in enumerate(core_imgs[c]):
            for t in range(TPI):
                r0 = ST * t
                nr = min(ST, H2R - r0)
                blk = lin[il][:, t * 10:(t + 1) * 10].reshape(n2s, ST, 10)
                for ko, co in enumerate(live2[b]):
                    out_full[b, co, r0:r0 + nr, :] = blk[ko, :nr, :] \
                        + bln[None, :]
    return out_full


# revision 4
# speedup vs baseline: 1.0149x; 1.0125x over previous
"""ALSH ConvNet Trainium2 kernel v2: mask-aware channel-sparse tiling.

Host computes the ALSH hashes (layer-1 query hash from x patch sums;
layer-2 query hash from a host conv1 — the ALSH query needs full-conv1
patch sums regardless), yielding per-image live channel sets:
  live1[b] = channels with fh1 == qh1[b]   (conv1 output channels)
  live2[b] = channels with fh2 == qh2[b]   (conv2 output channels)
Dead channels are provably zero downstream (mask commutes with relu and
the linear), so the device only computes live channels; masked output
rows are just bl (host fills).

Device program (SPMD, one core = IPC images x TPI row-tiles):
  conv1 banded matmul (K=3ci x (UW+2) x-rows, M=n1 x UW h1-rows, stream
  w) -> relu drain -> conv2 TRANSPOSED (stationary = h1 slab columns,
  moving = per-image banded W2 consts; out[w, (co,hr)]) -> relu drain
  -> fused transposed linear (stationary = h2T, moving = Wl chunks,
  N=10) -> out [n2*stride, 10] per tile. Host scatters + bl.
"""
import numpy as np
import concourse.bass as bass
from concourse import bacc
import concourse.tile as tile
import concourse.mybir as mybir
from concourse.bass_utils import run_bass_kernel_spmd

f32 = mybir.dt.float32
f16 = mybir.dt.float16
AF = mybir.ActivationFunctionType
ALU = mybir.AluOpType

R = 0.1
U = 0.99
N_CORES = 8
H, WX = 260, 1004
H1R, W1W = 258, 1002   # conv1 out rows/cols
H2R, W2W = 256, 1000   # conv2 out rows/cols
W1CH = [(0, 512), (512, 490)]          # conv1 psum column chunks
LKS = [(k * 128, 128) for k in range(7)] + [(896, 104)]  # w chunks


def _filter_hash(W, a, b):
    Cout = W.shape[0]
    wf = W.reshape(Cout, -1).astype(np.float32)
    norms = np.sqrt((wf * wf).sum(1))
    ws = wf * np.float32(U / norms.max())
    n2 = (ws * ws).sum(1)
    powers = np.stack([n2, n2**2, n2**4, n2**8, n2**16], axis=1)
    Pw = np.concatenate([ws, powers], axis=1).astype(np.float32)
    return np.mod(np.floor(
        (Pw @ a.astype(np.float32) + np.float32(b)) / np.float32(R)
    ).astype(np.int64), 2).astype(np.int64)


def _qhash(q, a, b):
    qn = q / np.maximum(np.linalg.norm(q, axis=1, keepdims=True), 1e-12)
    v = qn @ a[:q.shape[1]].astype(np.float64) \
        + 0.5 * a[q.shape[1]:].astype(np.float64).sum() + float(b)
    return np.mod(np.floor(v / R).astype(np.int64), 2)


def _build_l1(W1g, n1s, UW):
    # [3ci*(UW+2), 3dj * (n1s*UW)]; col (dj, co, u), row (ci, dh):
    # value = W1g[co, ci, dh-u, dj] for 0 <= dh-u < 3
    KH = UW + 2
    M = n1s * UW
    L = np.zeros((3 * KH, 3 * M), np.float32)
    n1 = W1g.shape[0]
    for dj in range(3):
        for co in range(n1):
            for ci in range(3):
                for u in range(UW):
                    for di in range(3):
                        L[ci * KH + u + di, dj * M + co * UW + u] = \
                            W1g[co, ci, di, dj]
    return L


def _build_w2(W2g, n1s, n2s, UW, ST):
    # [n1s*UW, 3dw * (n2s*ST)]; row (ci, u), col (dw, co, hr):
    # value = W2g[co, ci, u-hr, dw] for 0 <= u-hr < 3
    F = n2s * ST
    Bm = np.zeros((n1s * UW, 3 * F), np.float32)
    n2, n1 = W2g.shape[0], W2g.shape[1]
    for dw in range(3):
        for co in range(n2):
            for ci in range(n1):
                for hr in range(ST):
                    for dh in range(3):
                        u = hr + dh
                        if u < UW:
                            Bm[ci * UW + u, dw * F + co * ST + hr] = \
                                W2g[co, ci, dh, dw]
    return Bm


def _build_wlt(Wln):
    WlT = Wln.T.astype(np.float32)             # [1000, 10]
    wltf = np.zeros((128, 80), np.float32)
    for k, (k0, K) in enumerate(LKS):
        wltf[:K, k * 10:k * 10 + 10] = WlT[k0:k0 + K]
    return wltf


def _build_nc(TPI, IPC, UW, n1s, n2s):
    """TPI tiles/image, IPC images/core, UW-row h1 window (stride UW-2),
    n1s/n2s live-channel slots."""
    ST = UW - 2
    KH = UW + 2
    K1 = 3 * KH                 # conv1 contraction rows (x slab)
    M1 = n1s * UW               # conv1 out partitions
    K2 = n1s * UW               # conv2 contraction rows (h1 slab)
    F2 = n2s * ST               # conv2 free (co, hr)
    CPB = max(1, 512 // F2)     # conv2 chunks per psum bank
    NB = (len(LKS) + CPB - 1) // CPB   # banks per tile
    TPC = TPI * IPC

    nc = bacc.Bacc("TRN2", target_bir_lowering=False)
    xsP = nc.declare_dram_parameter("xs", [TPC, K1, WX], f16, isOutput=False)
    bootP = nc.declare_dram_parameter("boot", [K1, WX + 3 * M1], f16,
                                      isOutput=False)
    l1P = nc.declare_dram_parameter("l1", [IPC, K1, 3 * M1], f16,
                                    isOutput=False)
    w2P = nc.declare_dram_parameter("w2", [IPC, K2, 3 * F2], f16,
                                    isOutput=False)
    wltP = nc.declare_dram_parameter("wlt", [128, 80], f16, isOutput=False)
    outP = nc.declare_dram_parameter("out", [IPC, F2, TPI * 10], f32,
                                     isOutput=True)
    STL = H2R - ST * (TPI - 1)      # valid out rows in the tail tile

    with tile.TileContext(nc) as tc:
        with tc.tile_pool(name="consts", bufs=1) as cpool, \
             tc.tile_pool(name="imc", bufs=4) as imc, \
             tc.tile_pool(name="xcp", bufs=5) as xcp, \
             tc.tile_pool(name="h1p", bufs=4) as h1p, \
             tc.tile_pool(name="h2p", bufs=6) as h2p, \
             tc.tile_pool(name="outp", bufs=2) as outp, \
             tc.tile_pool(name="c1ps", bufs=3, space="PSUM") as c1ps, \
             tc.tile_pool(name="c2ps", bufs=3, space="PSUM") as c2ps, \
             tc.tile_pool(name="lps", bufs=2, space="PSUM") as lps:

            l1s, w2s = {}, {}
            xcs, h1s, h2s = {}, {}, {}

            def load_x(ti):
                xc = xcp.tile([K1, WX], f16, tag="xc")
                nc.sync.dma_start(xc[:], xsP[ti])
                xcs[ti] = xc

            def load_img_consts(im, split=False):
                if split:
                    # one DMA: tile-0 x slab + img0 conv1 weights (same
                    # 126-partition layout) -> first matmul waits on a
                    # single DMA chain
                    bt = imc.tile([K1, WX + 3 * M1], f16, tag="boot")
                    nc.sync.dma_start(bt[:], bootP[:])
                    xcs[0] = bt
                    l1s[0] = bt[:, WX:WX + 3 * M1]
                else:
                    t1 = imc.tile([K1, 3 * M1], f16, tag="l1d")
                    nc.sync.dma_start(t1[:], l1P[im])
                    l1s[im] = t1
                if split:
                    load_x(1)
                t2 = imc.tile([K2, 3 * F2], f16, tag="w2d")
                nc.sync.dma_start(t2[:], w2P[im])
                w2s[im] = t2

            # PE warm-up during the initial DMA latency: zeros via gpsimd
            # memset, then throwaway matmuls so the p-state ramp completes
            # before real work arrives. Uses an lps psum generation that is
            # recycled long before the first real linear group.
            wz = cpool.tile([128, 16], f16, tag="wz")
            nc.gpsimd.memset(wz[:], 0.0)
            wps = lps.tile([8, 512], f32, tag="lps")
            for _ in range(1):
                nc.tensor.matmul(wps[0:8, 0:16], wz[:, 0:8], wz[:, 0:16],
                                 start=True, stop=True, skip_group_check=True)

            load_img_consts(0, split=True)

            # wlt: DMA then DVE copy (engine-produced matmul operand);
            # first needed by linear at ti=2, so loaded after the hot path.
            wlts = cpool.tile([128, 80], f16, tag="wlt_d")
            nc.sync.dma_start(wlts[:], wltP[:])

            def conv1(ti):
                im = ti // TPI
                h1 = h1p.tile([M1, W1W], f16, tag="h1")
                xc = xcs.pop(ti)
                for wi, (w0, N) in enumerate(W1CH):
                    ps = c1ps.tile([M1, 512], f32, tag="c1ps")
                    for dj in range(3):
                        if isinstance(xc, tuple):
                            src = xc[wi][:, dj:dj + N] if wi == 0                                 else xc[1][:, dj:dj + N]
                        else:
                            src = xc[:, w0 + dj:w0 + dj + N]
                        nc.tensor.matmul(
                            ps[0:M1, 0:N],
                            l1s[im][:, dj * M1:(dj + 1) * M1],
                            src,
                            start=(dj == 0), stop=(dj == 2))
                    if wi == 0:
                        nc.scalar.activation(h1[:, w0:w0 + N], ps[0:M1, 0:N],
                                             AF.Relu)
                    else:
                        nc.vector.tensor_scalar_max(h1[:, w0:w0 + N],
                                                    ps[0:M1, 0:N], 0.0)
                h1s[ti] = h1

            def conv2(ti):
                im, t = divmod(ti, TPI)
                tail = (t == TPI - 1)
                STt = STL if tail else ST
                h1 = h1s.pop(ti)
                h2bs = []
                for b in range(NB):
                    ks = range(b * CPB, min((b + 1) * CPB, len(LKS)))
                    h2 = h2p.tile([128, len(ks) * F2], f16, tag="h2")
                    h2bs.append(h2)
                    ps = c2ps.tile([128, 512], f32, tag="c2ps")
                    for ci_, k in enumerate(ks):
                        w0, M = LKS[k]
                        for dw in range(3):
                            rhs = w2s[im][:, dw * F2:(dw + 1) * F2]
                            dst = ps[0:M, ci_ * F2:ci_ * F2 + F2]
                            if tail:
                                rhs = rhs.rearrange(
                                    "p (c h) -> p c h", c=n2s)[:, :, 0:STt]
                                dst = dst.rearrange(
                                    "p (c h) -> p c h", c=n2s)[:, :, 0:STt]
                            nc.tensor.matmul(
                                dst, h1[:, w0 + dw:w0 + dw + M], rhs,
                                start=(ci_ == 0 and dw == 0),
                                stop=(ci_ == len(ks) - 1 and dw == 2),
                                skip_group_check=True)
                    ncols = len(ks) * F2
                    if tail:
                        # skip dead hr columns in the tail tile's drains
                        dst = h2[:, 0:ncols].rearrange(
                            "p (c f) -> p c f", c=len(ks) * n2s)[:, :, 0:STt]
                        src = ps[0:128, 0:ncols].rearrange(
                            "p (c f) -> p c f", c=len(ks) * n2s)[:, :, 0:STt]
                    else:
                        dst = h2[:, 0:ncols]
                        src = ps[0:128, 0:ncols]
                    if b % 2 == 0:
                        nc.scalar.activation(dst, src, AF.Relu)
                    else:
                        nc.vector.tensor_scalar_max(dst, src, 0.0)
                h2s[ti] = h2bs

            pls, obs = {}, {}

            def linear(ti):
                im, t = divmod(ti, TPI)
                h2bs = h2s.pop(ti)
                if t == 0:
                    pls[im] = lps.tile([F2, TPI * 10], f32, name="plin",
                                       tag="lps")
                pl = pls[im]
                for k, (k0, K) in enumerate(LKS):
                    h2 = h2bs[k // CPB]
                    kk = k % CPB
                    nc.tensor.matmul(pl[0:F2, t * 10:t * 10 + 10],
                                     h2[0:K, kk * F2:kk * F2 + F2],
                                     wlts[0:K, k * 10:k * 10 + 10],
                                     start=(t == 0 and k == 0),
                                     stop=(t == TPI - 1 and k == len(LKS) - 1),
                                     skip_group_check=True)
                if t == TPI - 1:
                    pl = pls.pop(im)
                    ob = outp.tile([F2, TPI * 10], f32, tag="outsb")
                    nc.vector.tensor_copy(ob[:], pl[0:F2, :])
                    nc.sync.dma_start(outP[im], ob[:])

            # software pipeline: conv1(t) | conv2(t-1) | linear(t-2)
            for ti in range(TPC + 2):
                if ti < TPC:
                    if ti % TPI == 0 and ti // TPI + 1 < IPC:
                        load_img_consts(ti // TPI + 1)
                    conv1(ti)
                    if ti + 2 < TPC:
                        load_x(ti + 2)
                if 1 <= ti <= TPC:
                    conv2(ti - 1)
                if 2 <= ti:
                    linear(ti - 2)
    nc.compile()
    return nc


_CACHE = {}
LAST_RES = None


def _host_conv1(x, W1g):
    # relu(conv1) for gathered live channels; x [B,3,260,1004] f32,
    # W1g [B, n1s, 3, 3, 3] per-image gathered weights (zero-padded).
    from numpy.lib.stride_tricks import sliding_window_view
    win = sliding_window_view(x, (3, 3), axis=(2, 3))  # [B,3,258,1002,3,3]
    h = np.einsum("bchwij,bkcij->bkhw", win, W1g, optimize=True)
    return np.maximum(h, 0.0, out=h)


def kernel(x, W1, b1, W2, a1, a2, b2, Wl, bl, **kw):
    x = np.asarray(x, np.float32)
    W1n = np.asarray(W1, np.float32)
    W2n = np.asarray(W2, np.float32)
    a1n = np.asarray(a1, np.float32)
    a2n = np.asarray(a2, np.float32)
    b1n = float(np.asarray(b1, np.float32))
    b2n = float(np.asarray(b2, np.float32))
    Wln = np.asarray(Wl, np.float32)
    bln = np.asarray(bl, np.float32)
    B = x.shape[0]

    # ---- host: ALSH hashes -> per-image live channel sets ----
    fh1 = _filter_hash(W1n, a1n, b1n)
    fh2 = _filter_hash(W2n, a2n, b2n)
    q1v = np.empty((B, 27), np.float64)
    for i in range(3):
        for j in range(3):
            s = x[:, :, i:i + H1R, j:j + W1W].sum(axis=(2, 3),
                                                  dtype=np.float64)
            for ci in range(3):
                q1v[:, ci * 9 + i * 3 + j] = s[:, ci]
    qh1 = _qhash(q1v, a1n, b1n)
    mask1 = (fh1[None, :] == qh1[:, None])              # [B, 5] bool
    live1 = [np.where(mask1[b])[0] for b in range(B)]
    n1s = max(1, max(len(v) for v in live1))

    # gathered conv1 weights (zero-padded to n1s slots)
    W1g = np.zeros((B, n1s, 3, 3, 3), np.float32)
    for b in range(B):
        for k, c in enumerate(live1[b]):
            W1g[b, k] = W1n[c]

    # layer-2 query hash needs full-conv1 patch sums -> host conv1
    h1h = _host_conv1(x, W1g)                           # [B, n1s, 258, 1002]
    q2v = np.zeros((B, 45), np.float64)
    for i in range(3):
        for j in range(3):
            s = h1h[:, :, i:i + H2R, j:j + W2W].sum(axis=(2, 3),
                                                    dtype=np.float64)
            for b in range(B):
                for k, c in enumerate(live1[b]):
                    q2v[b, c * 9 + i * 3 + j] = s[b, k]
    qh2 = _qhash(q2v, a2n, b2n)
    mask2 = (fh2[None, :] == qh2[:, None])              # [B, 5] bool
    live2 = [np.where(mask2[b])[0] if len(live1[b]) else np.empty(0, np.int64)
             for b in range(B)]
    n2s = max(1, max((len(v) for v in live2), default=1))

    out_full = np.broadcast_to(
        bln[None, None, None, :], (B, 5, H2R, 10)).astype(np.float32).copy()

    work = [b for b in range(B) if len(live1[b]) and len(live2[b])]
    if not work:
        return out_full

    # ---- geometry ----
    UW = min(40, 128 // n1s)       # h1 slab rows (K2 = n1s*UW <= 128 etc.)
    ST = UW - 2
    TPI = -(-H2R // ST)            # tiles per image
    IPC = -(-len(work) // N_CORES)  # images per core (padded)
    K1 = 3 * (UW + 2)
    M1 = n1s * UW
    F2 = n2s * ST

    key = (TPI, IPC, UW, n1s, n2s)
    if key not in _CACHE:
        _CACHE.clear()
        _CACHE[key] = _build_nc(TPI, IPC, UW, n1s, n2s)
    nc = _CACHE[key]

    # ---- host prep: per-core inputs ----
    x16 = x.astype(np.float16)
    wltf = _build_wlt(Wln).astype(np.float16)
    TPC = TPI * IPC
    in_maps = []
    core_imgs = []
    for c in range(N_CORES):
        imgs = work[c * IPC:(c + 1) * IPC]
        core_imgs.append(imgs)
        xs = np.zeros((TPC, K1, WX), np.float16)
        l1c = np.zeros((IPC, K1, 3 * M1), np.float16)
        w2c = np.zeros((IPC, K2loc := n1s * UW, 3 * F2), np.float16)
        for il, b in enumerate(imgs):
            W2g = np.zeros((n2s, n1s, 3, 3), np.float32)
            for ko, co in enumerate(live2[b]):
                for ki, ci in enumerate(live1[b]):
                    W2g[ko, ki] = W2n[co, ci]
            l1c[il] = _build_l1(W1g[b], n1s, UW).astype(np.float16)
            w2c[il] = _build_w2(W2g, n1s, n2s, UW, ST).astype(np.float16)
            for t in range(TPI):
                r0 = ST * t
                nrow = min(UW + 2, H - r0)
                sl = x16[b, :, r0:r0 + nrow, :]           # [3, nrow, 1004]
                dst = xs[il * TPI + t].reshape(3, UW + 2, WX)
                dst[:, :nrow, :] = sl
        boot = np.concatenate([xs[0], l1c[0]], axis=1)
        in_maps.append({"xs": xs, "boot": boot, "l1": l1c, "w2": w2c,
                        "wlt": wltf})

    res = run_bass_kernel_spmd(nc, in_maps, core_ids=list(range(N_CORES)),
                               **kw)
    global LAST_RES
    LAST_RES = res

    # ---- host: scatter ----
    for c in range(N_CORES):
        lin = res.results[c]["out"]                     # [IPC, F2, TPI*10]
        for il, b in enumerate(core_imgs[c]):
            for t in range(TPI):
                r0 = ST * t
                nr = min(ST, H2R - r0)
                blk = lin[il][:, t * 10:(t + 1) * 10].reshape(n2s, ST, 10)
                for ko, co in enumerate(live2[b]):
                    out_full[b, co, r0:r0 + nr, :] = blk[ko, :nr, :] \
                        + bln[None, :]
    return out_full


# revision 5
# speedup vs baseline: 1.0178x; 1.0029x over previous
"""ALSH ConvNet Trainium2 kernel v2: mask-aware channel-sparse tiling.

Host computes the ALSH hashes (layer-1 query hash from x patch sums;
layer-2 query hash from a host conv1 — the ALSH query needs full-conv1
patch sums regardless), yielding per-image live channel sets:
  live1[b] = channels with fh1 == qh1[b]   (conv1 output channels)
  live2[b] = channels with fh2 == qh2[b]   (conv2 output channels)
Dead channels are provably zero downstream (mask commutes with relu and
the linear), so the device only computes live channels; masked output
rows are just bl (host fills).

Device program (SPMD, one core = IPC images x TPI row-tiles):
  conv1 banded matmul (K=3ci x (UW+2) x-rows, M=n1 x UW h1-rows, stream
  w) -> relu drain -> conv2 TRANSPOSED (stationary = h1 slab columns,
  moving = per-image banded W2 consts; out[w, (co,hr)]) -> relu drain
  -> fused transposed linear (stationary = h2T, moving = Wl chunks,
  N=10) -> out [n2*stride, 10] per tile. Host scatters + bl.
"""
import numpy as np
import concourse.bass as bass
from concourse import bacc
import concourse.tile as tile
import concourse.mybir as mybir
from concourse.bass_utils import run_bass_kernel_spmd

f32 = mybir.dt.float32
f16 = mybir.dt.float16
AF = mybir.ActivationFunctionType
ALU = mybir.AluOpType

R = 0.1
U = 0.99
N_CORES = 8
H, WX = 260, 1004
H1R, W1W = 258, 1002   # conv1 out rows/cols
H2R, W2W = 256, 1000   # conv2 out rows/cols
W1CH = [(0, 512), (512, 490)]          # conv1 psum column chunks
LKS = [(k * 128, 128) for k in range(7)] + [(896, 104)]  # w chunks


def _filter_hash(W, a, b):
    Cout = W.shape[0]
    wf = W.reshape(Cout, -1).astype(np.float32)
    norms = np.sqrt((wf * wf).sum(1))
    ws = wf * np.float32(U / norms.max())
    n2 = (ws * ws).sum(1)
    powers = np.stack([n2, n2**2, n2**4, n2**8, n2**16], axis=1)
    Pw = np.concatenate([ws, powers], axis=1).astype(np.float32)
    return np.mod(np.floor(
        (Pw @ a.astype(np.float32) + np.float32(b)) / np.float32(R)
    ).astype(np.int64), 2).astype(np.int64)


def _qhash(q, a, b):
    qn = q / np.maximum(np.linalg.norm(q, axis=1, keepdims=True), 1e-12)
    v = qn @ a[:q.shape[1]].astype(np.float64) \
        + 0.5 * a[q.shape[1]:].astype(np.float64).sum() + float(b)
    return np.mod(np.floor(v / R).astype(np.int64), 2)


def _build_l1(W1g, n1s, UW):
    # [3ci*(UW+2), 3dj * (n1s*UW)]; col (dj, co, u), row (ci, dh):
    # value = W1g[co, ci, dh-u, dj] for 0 <= dh-u < 3
    KH = UW + 2
    M = n1s * UW
    L = np.zeros((3 * KH, 3 * M), np.float32)
    n1 = W1g.shape[0]
    for dj in range(3):
        for co in range(n1):
            for ci in range(3):
                for u in range(UW):
                    for di in range(3):
                        L[ci * KH + u + di, dj * M + co * UW + u] = \
                            W1g[co, ci, di, dj]
    return L


def _build_w2(W2g, n1s, n2s, UW, ST):
    # [n1s*UW, 3dw * (n2s*ST)]; row (ci, u), col (dw, co, hr):
    # value = W2g[co, ci, u-hr, dw] for 0 <= u-hr < 3
    F = n2s * ST
    Bm = np.zeros((n1s * UW, 3 * F), np.float32)
    n2, n1 = W2g.shape[0], W2g.shape[1]
    for dw in range(3):
        for co in range(n2):
            for ci in range(n1):
                for hr in range(ST):
                    for dh in range(3):
                        u = hr + dh
                        if u < UW:
                            Bm[ci * UW + u, dw * F + co * ST + hr] = \
                                W2g[co, ci, dh, dw]
    return Bm


def _build_wlt(Wln):
    WlT = Wln.T.astype(np.float32)             # [1000, 10]
    wltf = np.zeros((128, 80), np.float32)
    for k, (k0, K) in enumerate(LKS):
        wltf[:K, k * 10:k * 10 + 10] = WlT[k0:k0 + K]
    return wltf


def _build_nc(TPI, IPC, UW, n1s, n2s):
    """TPI tiles/image, IPC images/core, UW-row h1 window (stride UW-2),
    n1s/n2s live-channel slots."""
    ST = UW - 2
    KH = UW + 2
    K1 = 3 * KH                 # conv1 contraction rows (x slab)
    M1 = n1s * UW               # conv1 out partitions
    K2 = n1s * UW               # conv2 contraction rows (h1 slab)
    F2 = n2s * ST               # conv2 free (co, hr)
    CPB = max(1, 512 // F2)     # conv2 chunks per psum bank
    NB = (len(LKS) + CPB - 1) // CPB   # banks per tile
    TPC = TPI * IPC

    nc = bacc.Bacc("TRN2", target_bir_lowering=False)
    xsP = nc.declare_dram_parameter("xs", [TPC, K1, WX], f16, isOutput=False)
    bootP = nc.declare_dram_parameter("boot", [K1, WX + 3 * M1], f16,
                                      isOutput=False)
    l1P = nc.declare_dram_parameter("l1", [IPC, K1, 3 * M1], f16,
                                    isOutput=False)
    w2P = nc.declare_dram_parameter("w2", [IPC, K2, 3 * F2], f16,
                                    isOutput=False)
    wltP = nc.declare_dram_parameter("wlt", [128, 80], f16, isOutput=False)
    outP = nc.declare_dram_parameter("out", [IPC, F2, TPI * 10], f32,
                                     isOutput=True)
    STL = H2R - ST * (TPI - 1)      # valid out rows in the tail tile

    with tile.TileContext(nc) as tc:
        with tc.tile_pool(name="consts", bufs=1) as cpool, \
             tc.tile_pool(name="imc", bufs=4) as imc, \
             tc.tile_pool(name="xcp", bufs=5) as xcp, \
             tc.tile_pool(name="h1p", bufs=4) as h1p, \
             tc.tile_pool(name="h2p", bufs=6) as h2p, \
             tc.tile_pool(name="outp", bufs=2) as outp, \
             tc.tile_pool(name="c1ps", bufs=4, space="PSUM") as c1ps, \
             tc.tile_pool(name="c2ps", bufs=3, space="PSUM") as c2ps, \
             tc.tile_pool(name="lps", bufs=1, space="PSUM") as lps:

            l1s, w2s = {}, {}
            xcs, h1s, h2s = {}, {}, {}

            def load_x(ti):
                xc = xcp.tile([K1, WX], f16, tag="xc")
                nc.sync.dma_start(xc[:], xsP[ti])
                xcs[ti] = xc

            def load_img_consts(im, split=False):
                if split:
                    # one DMA: tile-0 x slab + img0 conv1 weights (same
                    # 126-partition layout) -> first matmul waits on a
                    # single DMA chain
                    bt = imc.tile([K1, WX + 3 * M1], f16, tag="boot")
                    nc.sync.dma_start(bt[:], bootP[:])
                    xcs[0] = bt
                    l1s[0] = bt[:, WX:WX + 3 * M1]
                else:
                    t1 = imc.tile([K1, 3 * M1], f16, tag="l1d")
                    nc.sync.dma_start(t1[:], l1P[im])
                    l1s[im] = t1
                if split:
                    load_x(1)
                t2 = imc.tile([K2, 3 * F2], f16, tag="w2d")
                nc.sync.dma_start(t2[:], w2P[im])
                w2s[im] = t2

            # PE warm-up during the initial DMA latency: zeros via gpsimd
            # memset, then throwaway matmuls so the p-state ramp completes
            # before real work arrives. Uses an lps psum generation that is
            # recycled long before the first real linear group.
            wz = cpool.tile([128, 16], f16, tag="wz")
            nc.gpsimd.memset(wz[:], 0.0)
            wps = lps.tile([8, 512], f32, tag="lps")
            for _ in range(1):
                nc.tensor.matmul(wps[0:8, 0:16], wz[:, 0:8], wz[:, 0:16],
                                 start=True, stop=True, skip_group_check=True)

            load_img_consts(0, split=True)

            # wlt: DMA then DVE copy (engine-produced matmul operand);
            # first needed by linear at ti=2, so loaded after the hot path.
            wlts = cpool.tile([128, 80], f16, tag="wlt_d")
            nc.sync.dma_start(wlts[:], wltP[:])

            def conv1(ti):
                im = ti // TPI
                h1 = h1p.tile([M1, W1W], f16, tag="h1")
                xc = xcs.pop(ti)
                for wi, (w0, N) in enumerate(W1CH):
                    ps = c1ps.tile([M1, 512], f32, tag="c1ps")
                    for dj in range(3):
                        if isinstance(xc, tuple):
                            src = xc[wi][:, dj:dj + N] if wi == 0                                 else xc[1][:, dj:dj + N]
                        else:
                            src = xc[:, w0 + dj:w0 + dj + N]
                        nc.tensor.matmul(
                            ps[0:M1, 0:N],
                            l1s[im][:, dj * M1:(dj + 1) * M1],
                            src,
                            start=(dj == 0), stop=(dj == 2))
                    if wi == 0:
                        nc.scalar.activation(h1[:, w0:w0 + N], ps[0:M1, 0:N],
                                             AF.Relu)
                    else:
                        nc.vector.tensor_scalar_max(h1[:, w0:w0 + N],
                                                    ps[0:M1, 0:N], 0.0)
                h1s[ti] = h1

            def conv2(ti):
                im, t = divmod(ti, TPI)
                tail = (t == TPI - 1)
                STt = STL if tail else ST
                h1 = h1s.pop(ti)
                h2bs = []
                for b in range(NB):
                    ks = range(b * CPB, min((b + 1) * CPB, len(LKS)))
                    h2 = h2p.tile([128, len(ks) * F2], f16, tag="h2")
                    h2bs.append(h2)
                    ps = c2ps.tile([128, 512], f32, tag="c2ps")
                    for ci_, k in enumerate(ks):
                        w0, M = LKS[k]
                        for dw in range(3):
                            rhs = w2s[im][:, dw * F2:(dw + 1) * F2]
                            dst = ps[0:M, ci_ * F2:ci_ * F2 + F2]
                            if tail:
                                rhs = rhs.rearrange(
                                    "p (c h) -> p c h", c=n2s)[:, :, 0:STt]
                                dst = dst.rearrange(
                                    "p (c h) -> p c h", c=n2s)[:, :, 0:STt]
                            nc.tensor.matmul(
                                dst, h1[:, w0 + dw:w0 + dw + M], rhs,
                                start=(ci_ == 0 and dw == 0),
                                stop=(ci_ == len(ks) - 1 and dw == 2),
                                skip_group_check=True)
                    ncols = len(ks) * F2
                    if tail:
                        # skip dead hr columns in the tail tile's drains
                        dst = h2[:, 0:ncols].rearrange(
                            "p (c f) -> p c f", c=len(ks) * n2s)[:, :, 0:STt]
                        src = ps[0:128, 0:ncols].rearrange(
                            "p (c f) -> p c f", c=len(ks) * n2s)[:, :, 0:STt]
                    else:
                        dst = h2[:, 0:ncols]
                        src = ps[0:128, 0:ncols]
                    if b % 2 == 0:
                        nc.scalar.activation(dst, src, AF.Relu)
                    else:
                        nc.vector.tensor_scalar_max(dst, src, 0.0)
                h2s[ti] = h2bs

            pls, obs = {}, {}

            def linear(ti):
                im, t = divmod(ti, TPI)
                h2bs = h2s.pop(ti)
                if t == 0:
                    pls[im] = lps.tile([F2, TPI * 10], f32, name="plin",
                                       tag="lps")
                pl = pls[im]
                for k, (k0, K) in enumerate(LKS):
                    h2 = h2bs[k // CPB]
                    kk = k % CPB
                    nc.tensor.matmul(pl[0:F2, t * 10:t * 10 + 10],
                                     h2[0:K, kk * F2:kk * F2 + F2],
                                     wlts[0:K, k * 10:k * 10 + 10],
                                     start=(t == 0 and k == 0),
                                     stop=(t == TPI - 1 and k == len(LKS) - 1),
                                     skip_group_check=True)
                if t == TPI - 1:
                    pl = pls.pop(im)
                    ob = outp.tile([F2, TPI * 10], f32, tag="outsb")
                    nc.vector.tensor_copy(ob[:], pl[0:F2, :])
                    nc.sync.dma_start(outP[im], ob[:])

            # software pipeline: conv1(t) | conv2(t-1) | linear(t-2)
            for ti in range(TPC + 2):
                if ti < TPC:
                    if ti % TPI == 0 and ti // TPI + 1 < IPC:
                        load_img_consts(ti // TPI + 1)
                    conv1(ti)
                    if ti + 2 < TPC:
                        load_x(ti + 2)
                if 1 <= ti <= TPC:
                    conv2(ti - 1)
                if 2 <= ti:
                    linear(ti - 2)
    nc.compile()
    return nc


_CACHE = {}
LAST_RES = None


def _host_conv1(x, W1g):
    # relu(conv1) for gathered live channels; x [B,3,260,1004] f32,
    # W1g [B, n1s, 3, 3, 3] per-image gathered weights (zero-padded).
    from numpy.lib.stride_tricks import sliding_window_view
    win = sliding_window_view(x, (3, 3), axis=(2, 3))  # [B,3,258,1002,3,3]
    h = np.einsum("bchwij,bkcij->bkhw", win, W1g, optimize=True)
    return np.maximum(h, 0.0, out=h)


def kernel(x, W1, b1, W2, a1, a2, b2, Wl, bl, **kw):
    x = np.asarray(x, np.float32)
    W1n = np.asarray(W1, np.float32)
    W2n = np.asarray(W2, np.float32)
    a1n = np.asarray(a1, np.float32)
    a2n = np.asarray(a2, np.float32)
    b1n = float(np.asarray(b1, np.float32))
    b2n = float(np.asarray(b2, np.float32))
    Wln = np.asarray(Wl, np.float32)
    bln = np.asarray(bl, np.float32)
    B = x.shape[0]

    # ---- host: ALSH hashes -> per-image live channel sets ----
    fh1 = _filter_hash(W1n, a1n, b1n)
    fh2 = _filter_hash(W2n, a2n, b2n)
    q1v = np.empty((B, 27), np.float64)
    for i in range(3):
        for j in range(3):
            s = x[:, :, i:i + H1R, j:j + W1W].sum(axis=(2, 3),
                                                  dtype=np.float64)
            for ci in range(3):
                q1v[:, ci * 9 + i * 3 + j] = s[:, ci]
    qh1 = _qhash(q1v, a1n, b1n)
    mask1 = (fh1[None, :] == qh1[:, None])              # [B, 5] bool
    live1 = [np.where(mask1[b])[0] for b in range(B)]
    n1s = max(1, max(len(v) for v in live1))

    # gathered conv1 weights (zero-padded to n1s slots)
    W1g = np.zeros((B, n1s, 3, 3, 3), np.float32)
    for b in range(B):
        for k, c in enumerate(live1[b]):
            W1g[b, k] = W1n[c]

    # layer-2 query hash needs full-conv1 patch sums -> host conv1
    h1h = _host_conv1(x, W1g)                           # [B, n1s, 258, 1002]
    q2v = np.zeros((B, 45), np.float64)
    for i in range(3):
        for j in range(3):
            s = h1h[:, :, i:i + H2R, j:j + W2W].sum(axis=(2, 3),
                                                    dtype=np.float64)
            for b in range(B):
                for k, c in enumerate(live1[b]):
                    q2v[b, c * 9 + i * 3 + j] = s[b, k]
    qh2 = _qhash(q2v, a2n, b2n)
    mask2 = (fh2[None, :] == qh2[:, None])              # [B, 5] bool
    live2 = [np.where(mask2[b])[0] if len(live1[b]) else np.empty(0, np.int64)
             for b in range(B)]
    n2s = max(1, max((len(v) for v in live2), default=1))

    out_full = np.broadcast_to(
        bln[None, None, None, :], (B, 5, H2R, 10)).astype(np.float32).copy()

    work = [b for b in range(B) if len(live1[b]) and len(live2[b])]
    if not work:
        return out_full

    # ---- geometry ----
    UW = min(40, 128 // n1s)       # h1 slab rows (K2 = n1s*UW <= 128 etc.)
    ST = UW - 2
    TPI = -(-H2R // ST)            # tiles per image
    IPC = -(-len(work) // N_CORES)  # images per core (padded)
    K1 = 3 * (UW + 2)
    M1 = n1s * UW
    F2 = n2s * ST

    key = (TPI, IPC, UW, n1s, n2s)
    if key not in _CACHE:
        _CACHE.clear()
        _CACHE[key] = _build_nc(TPI, IPC, UW, n1s, n2s)
    nc = _CACHE[key]

    # ---- host prep: per-core inputs ----
    x16 = x.astype(np.float16)
    wltf = _build_wlt(Wln).astype(np.float16)
    TPC = TPI * IPC
    in_maps = []
    core_imgs = []
    for c in range(N_CORES):
        imgs = work[c * IPC:(c + 1) * IPC]
        core_imgs.append(imgs)
        xs = np.zeros((TPC, K1, WX), np.float16)
        l1c = np.zeros((IPC, K1, 3 * M1), np.float16)
        w2c = np.zeros((IPC, K2loc := n1s * UW, 3 * F2), np.float16)
        for il, b in enumerate(imgs):
            W2g = np.zeros((n2s, n1s, 3, 3), np.float32)
            for ko, co in enumerate(live2[b]):
                for ki, ci in enumerate(live1[b]):
                    W2g[ko, ki] = W2n[co, ci]
            l1c[il] = _build_l1(W1g[b], n1s, UW).astype(np.float16)
            w2c[il] = _build_w2(W2g, n1s, n2s, UW, ST).astype(np.float16)
            for t in range(TPI):
                r0 = ST * t
                nrow = min(UW + 2, H - r0)
                sl = x16[b, :, r0:r0 + nrow, :]           # [3, nrow, 1004]
                dst = xs[il * TPI + t].reshape(3, UW + 2, WX)
                dst[:, :nrow, :] = sl
        boot = np.concatenate([xs[0], l1c[0]], axis=1)
        in_maps.append({"xs": xs, "boot": boot, "l1": l1c, "w2": w2c,
                        "wlt": wltf})

    res = run_bass_kernel_spmd(nc, in_maps, core_ids=list(range(N_CORES)),
                               **kw)
    global LAST_RES
    LAST_RES = res

    # ---- host: scatter ----
    for c in range(N_CORES):
        lin = res.results[c]["out"]                     # [IPC, F2, TPI*10]
        for il, b in enumerate(core_imgs[c]):
            for t in range(TPI):
                r0 = ST * t
                nr = min(ST, H2R - r0)
                blk = lin[il][:, t * 10:(t + 1) * 10].reshape(n2s, ST, 10)
                for ko, co in enumerate(live2[b]):
                    out_full[b, co, r0:r0 + nr, :] = blk[ko, :nr, :] \
                        + bln[None, :]
    return out_full


# revision 6
# speedup vs baseline: 1.0190x; 1.0012x over previous
"""ALSH ConvNet Trainium2 kernel v2: mask-aware channel-sparse tiling.

Host computes the ALSH hashes (layer-1 query hash from x patch sums;
layer-2 query hash from a host conv1 — the ALSH query needs full-conv1
patch sums regardless), yielding per-image live channel sets:
  live1[b] = channels with fh1 == qh1[b]   (conv1 output channels)
  live2[b] = channels with fh2 == qh2[b]   (conv2 output channels)
Dead channels are provably zero downstream (mask commutes with relu and
the linear), so the device only computes live channels; masked output
rows are just bl (host fills).

Device program (SPMD, one core = IPC images x TPI row-tiles):
  conv1 banded matmul (K=3ci x (UW+2) x-rows, M=n1 x UW h1-rows, stream
  w) -> relu drain -> conv2 TRANSPOSED (stationary = h1 slab columns,
  moving = per-image banded W2 consts; out[w, (co,hr)]) -> relu drain
  -> fused transposed linear (stationary = h2T, moving = Wl chunks,
  N=10) -> out [n2*stride, 10] per tile. Host scatters + bl.
"""
import numpy as np
import concourse.bass as bass
from concourse import bacc
import concourse.tile as tile
import concourse.mybir as mybir
from concourse.bass_utils import run_bass_kernel_spmd

f32 = mybir.dt.float32
f16 = mybir.dt.float16
AF = mybir.ActivationFunctionType
ALU = mybir.AluOpType

R = 0.1
U = 0.99
N_CORES = 8
H, WX = 260, 1004
H1R, W1W = 258, 1002   # conv1 out rows/cols
H2R, W2W = 256, 1000   # conv2 out rows/cols
W1CH = [(0, 512), (512, 490)]          # conv1 psum column chunks
LKS = [(k * 128, 128) for k in range(7)] + [(896, 104)]  # w chunks


def _filter_hash(W, a, b):
    Cout = W.shape[0]
    wf = W.reshape(Cout, -1).astype(np.float32)
    norms = np.sqrt((wf * wf).sum(1))
    ws = wf * np.float32(U / norms.max())
    n2 = (ws * ws).sum(1)
    powers = np.stack([n2, n2**2, n2**4, n2**8, n2**16], axis=1)
    Pw = np.concatenate([ws, powers], axis=1).astype(np.float32)
    return np.mod(np.floor(
        (Pw @ a.astype(np.float32) + np.float32(b)) / np.float32(R)
    ).astype(np.int64), 2).astype(np.int64)


def _qhash(q, a, b):
    qn = q / np.maximum(np.linalg.norm(q, axis=1, keepdims=True), 1e-12)
    v = qn @ a[:q.shape[1]].astype(np.float64) \
        + 0.5 * a[q.shape[1]:].astype(np.float64).sum() + float(b)
    return np.mod(np.floor(v / R).astype(np.int64), 2)


def _build_l1(W1g, n1s, UW):
    # [3ci*(UW+2), 3dj * (n1s*UW)]; col (dj, co, u), row (ci, dh):
    # value = W1g[co, ci, dh-u, dj] for 0 <= dh-u < 3
    KH = UW + 2
    M = n1s * UW
    L = np.zeros((3 * KH, 3 * M), np.float32)
    n1 = W1g.shape[0]
    for dj in range(3):
        for co in range(n1):
            for ci in range(3):
                for u in range(UW):
                    for di in range(3):
                        L[ci * KH + u + di, dj * M + co * UW + u] = \
                            W1g[co, ci, di, dj]
    return L


def _build_w2(W2g, n1s, n2s, UW, ST):
    # [n1s*UW, 3dw * (n2s*ST)]; row (ci, u), col (dw, co, hr):
    # value = W2g[co, ci, u-hr, dw] for 0 <= u-hr < 3
    F = n2s * ST
    Bm = np.zeros((n1s * UW, 3 * F), np.float32)
    n2, n1 = W2g.shape[0], W2g.shape[1]
    for dw in range(3):
        for co in range(n2):
            for ci in range(n1):
                for hr in range(ST):
                    for dh in range(3):
                        u = hr + dh
                        if u < UW:
                            Bm[ci * UW + u, dw * F + co * ST + hr] = \
                                W2g[co, ci, dh, dw]
    return Bm


def _build_wlt(Wln):
    WlT = Wln.T.astype(np.float32)             # [1000, 10]
    wltf = np.zeros((128, 80), np.float32)
    for k, (k0, K) in enumerate(LKS):
        wltf[:K, k * 10:k * 10 + 10] = WlT[k0:k0 + K]
    return wltf


def _build_nc(TPI, IPC, UW, n1s, n2s):
    """TPI tiles/image, IPC images/core, UW-row h1 window (stride UW-2),
    n1s/n2s live-channel slots."""
    ST = UW - 2
    KH = UW + 2
    K1 = 3 * KH                 # conv1 contraction rows (x slab)
    M1 = n1s * UW               # conv1 out partitions
    K2 = n1s * UW               # conv2 contraction rows (h1 slab)
    F2 = n2s * ST               # conv2 free (co, hr)
    CPB = max(1, 512 // F2)     # conv2 chunks per psum bank
    NB = (len(LKS) + CPB - 1) // CPB   # banks per tile
    TPC = TPI * IPC

    nc = bacc.Bacc("TRN2", target_bir_lowering=False)
    xsP = nc.declare_dram_parameter("xs", [TPC, K1, WX], f16, isOutput=False)
    bootP = nc.declare_dram_parameter("boot", [K1, WX + 3 * M1], f16,
                                      isOutput=False)
    l1P = nc.declare_dram_parameter("l1", [IPC, K1, 3 * M1], f16,
                                    isOutput=False)
    w2P = nc.declare_dram_parameter("w2", [IPC, K2, 3 * F2], f16,
                                    isOutput=False)
    wltP = nc.declare_dram_parameter("wlt", [128, 80], f16, isOutput=False)
    outP = nc.declare_dram_parameter("out", [IPC, F2, TPI * 10], f16,
                                     isOutput=True)
    STL = H2R - ST * (TPI - 1)      # valid out rows in the tail tile

    with tile.TileContext(nc) as tc:
        with tc.tile_pool(name="consts", bufs=1) as cpool, \
             tc.tile_pool(name="imc", bufs=4) as imc, \
             tc.tile_pool(name="xcp", bufs=5) as xcp, \
             tc.tile_pool(name="h1p", bufs=4) as h1p, \
             tc.tile_pool(name="h2p", bufs=6) as h2p, \
             tc.tile_pool(name="outp", bufs=2) as outp, \
             tc.tile_pool(name="c1ps", bufs=4, space="PSUM") as c1ps, \
             tc.tile_pool(name="c2ps", bufs=3, space="PSUM") as c2ps, \
             tc.tile_pool(name="lps", bufs=1, space="PSUM") as lps:

            l1s, w2s = {}, {}
            xcs, h1s, h2s = {}, {}, {}

            def load_x(ti):
                xc = xcp.tile([K1, WX], f16, tag="xc")
                nc.sync.dma_start(xc[:], xsP[ti])
                xcs[ti] = xc

            def load_img_consts(im, split=False):
                if split:
                    # one DMA: tile-0 x slab + img0 conv1 weights (same
                    # 126-partition layout) -> first matmul waits on a
                    # single DMA chain
                    bt = imc.tile([K1, WX + 3 * M1], f16, tag="boot")
                    nc.sync.dma_start(bt[:], bootP[:])
                    xcs[0] = bt
                    l1s[0] = [bt[:, WX + dj * M1:WX + (dj + 1) * M1]
                              for dj in range(3)]
                else:
                    t1 = imc.tile([K1, 3 * M1], f16, tag="l1d")
                    nc.sync.dma_start(t1[:], l1P[im])
                    l1s[im] = [t1[:, dj * M1:(dj + 1) * M1]
                               for dj in range(3)]
                if split:
                    load_x(1)
                t2 = imc.tile([K2, 3 * F2], f16, tag="w2d")
                nc.sync.dma_start(t2[:], w2P[im])
                w2s[im] = t2

            # PE warm-up during the initial DMA latency: zeros via gpsimd
            # memset, then throwaway matmuls so the p-state ramp completes
            # before real work arrives. Uses an lps psum generation that is
            # recycled long before the first real linear group.
            wz = cpool.tile([128, 16], f16, tag="wz")
            nc.gpsimd.memset(wz[:], 0.0)
            wps = lps.tile([8, 512], f32, tag="lps")
            for _ in range(1):
                nc.tensor.matmul(wps[0:8, 0:16], wz[:, 0:8], wz[:, 0:16],
                                 start=True, stop=True, skip_group_check=True)

            load_img_consts(0, split=True)

            # wlt: DMA then DVE copy (engine-produced matmul operand);
            # first needed by linear at ti=2, so loaded after the hot path.
            wlts = cpool.tile([128, 80], f16, tag="wlt_d")
            nc.sync.dma_start(wlts[:], wltP[:])

            def conv1(ti):
                im = ti // TPI
                h1 = h1p.tile([M1, W1W], f16, tag="h1")
                xc = xcs.pop(ti)
                for wi, (w0, N) in enumerate(W1CH):
                    ps = c1ps.tile([M1, 512], f32, tag="c1ps")
                    for dj in range(3):
                        if isinstance(xc, tuple):
                            src = xc[wi][:, dj:dj + N] if wi == 0                                 else xc[1][:, dj:dj + N]
                        else:
                            src = xc[:, w0 + dj:w0 + dj + N]
                        nc.tensor.matmul(
                            ps[0:M1, 0:N],
                            l1s[im][dj],
                            src,
                            start=(dj == 0), stop=(dj == 2))
                    if wi == 0:
                        nc.scalar.activation(h1[:, w0:w0 + N], ps[0:M1, 0:N],
                                             AF.Relu)
                    else:
                        nc.vector.tensor_scalar_max(h1[:, w0:w0 + N],
                                                    ps[0:M1, 0:N], 0.0)
                h1s[ti] = h1

            def conv2(ti):
                im, t = divmod(ti, TPI)
                tail = (t == TPI - 1)
                STt = STL if tail else ST
                h1 = h1s.pop(ti)
                h2bs = []
                for b in range(NB):
                    ks = range(b * CPB, min((b + 1) * CPB, len(LKS)))
                    h2 = h2p.tile([128, len(ks) * F2], f16, tag="h2")
                    h2bs.append(h2)
                    ps = c2ps.tile([128, 512], f32, tag="c2ps")
                    for ci_, k in enumerate(ks):
                        w0, M = LKS[k]
                        for dw in range(3):
                            rhs = w2s[im][:, dw * F2:(dw + 1) * F2]
                            dst = ps[0:M, ci_ * F2:ci_ * F2 + F2]
                            if tail:
                                rhs = rhs.rearrange(
                                    "p (c h) -> p c h", c=n2s)[:, :, 0:STt]
                                dst = dst.rearrange(
                                    "p (c h) -> p c h", c=n2s)[:, :, 0:STt]
                            nc.tensor.matmul(
                                dst, h1[:, w0 + dw:w0 + dw + M], rhs,
                                start=(ci_ == 0 and dw == 0),
                                stop=(ci_ == len(ks) - 1 and dw == 2),
                                skip_group_check=True)
                    ncols = len(ks) * F2
                    if tail:
                        # skip dead hr columns in the tail tile's drains
                        dst = h2[:, 0:ncols].rearrange(
                            "p (c f) -> p c f", c=len(ks) * n2s)[:, :, 0:STt]
                        src = ps[0:128, 0:ncols].rearrange(
                            "p (c f) -> p c f", c=len(ks) * n2s)[:, :, 0:STt]
                    else:
                        dst = h2[:, 0:ncols]
                        src = ps[0:128, 0:ncols]
                    if b % 2 == 0:
                        nc.scalar.activation(dst, src, AF.Relu)
                    else:
                        nc.vector.tensor_scalar_max(dst, src, 0.0)
                h2s[ti] = h2bs

            pls, obs = {}, {}

            def linear(ti):
                im, t = divmod(ti, TPI)
                h2bs = h2s.pop(ti)
                if t == 0:
                    pls[im] = lps.tile([F2, TPI * 10], f32, name="plin",
                                       tag="lps")
                pl = pls[im]
                for k, (k0, K) in enumerate(LKS):
                    h2 = h2bs[k // CPB]
                    kk = k % CPB
                    nc.tensor.matmul(pl[0:F2, t * 10:t * 10 + 10],
                                     h2[0:K, kk * F2:kk * F2 + F2],
                                     wlts[0:K, k * 10:k * 10 + 10],
                                     start=(t == 0 and k == 0),
                                     stop=(t == TPI - 1 and k == len(LKS) - 1),
                                     skip_group_check=True)
                if t == TPI - 1:
                    pl = pls.pop(im)
                    ob = outp.tile([F2, TPI * 10], f16, tag="outsb")
                    nc.vector.tensor_copy(ob[:], pl[0:F2, :])
                    nc.sync.dma_start(outP[im], ob[:])

            # software pipeline: conv1(t) | conv2(t-1) | linear(t-2)
            for ti in range(TPC + 2):
                if ti < TPC:
                    if ti % TPI == 0 and ti // TPI + 1 < IPC:
                        load_img_consts(ti // TPI + 1)
                    conv1(ti)
                    if ti + 2 < TPC:
                        load_x(ti + 2)
                if 1 <= ti <= TPC:
                    conv2(ti - 1)
                if 2 <= ti:
                    linear(ti - 2)
    nc.compile()
    return nc


_CACHE = {}
LAST_RES = None


def _host_conv1(x, W1g):
    # relu(conv1) for gathered live channels; x [B,3,260,1004] f32,
    # W1g [B, n1s, 3, 3, 3] per-image gathered weights (zero-padded).
    from numpy.lib.stride_tricks import sliding_window_view
    win = sliding_window_view(x, (3, 3), axis=(2, 3))  # [B,3,258,1002,3,3]
    h = np.einsum("bchwij,bkcij->bkhw", win, W1g, optimize=True)
    return np.maximum(h, 0.0, out=h)


def kernel(x, W1, b1, W2, a1, a2, b2, Wl, bl, **kw):
    x = np.asarray(x, np.float32)
    W1n = np.asarray(W1, np.float32)
    W2n = np.asarray(W2, np.float32)
    a1n = np.asarray(a1, np.float32)
    a2n = np.asarray(a2, np.float32)
    b1n = float(np.asarray(b1, np.float32))
    b2n = float(np.asarray(b2, np.float32))
    Wln = np.asarray(Wl, np.float32)
    bln = np.asarray(bl, np.float32)
    B = x.shape[0]

    # ---- host: ALSH hashes -> per-image live channel sets ----
    fh1 = _filter_hash(W1n, a1n, b1n)
    fh2 = _filter_hash(W2n, a2n, b2n)
    q1v = np.empty((B, 27), np.float64)
    for i in range(3):
        for j in range(3):
            s = x[:, :, i:i + H1R, j:j + W1W].sum(axis=(2, 3),
                                                  dtype=np.float64)
            for ci in range(3):
                q1v[:, ci * 9 + i * 3 + j] = s[:, ci]
    qh1 = _qhash(q1v, a1n, b1n)
    mask1 = (fh1[None, :] == qh1[:, None])              # [B, 5] bool
    live1 = [np.where(mask1[b])[0] for b in range(B)]
    n1s = max(1, max(len(v) for v in live1))

    # gathered conv1 weights (zero-padded to n1s slots)
    W1g = np.zeros((B, n1s, 3, 3, 3), np.float32)
    for b in range(B):
        for k, c in enumerate(live1[b]):
            W1g[b, k] = W1n[c]

    # layer-2 query hash needs full-conv1 patch sums -> host conv1
    h1h = _host_conv1(x, W1g)                           # [B, n1s, 258, 1002]
    q2v = np.zeros((B, 45), np.float64)
    for i in range(3):
        for j in range(3):
            s = h1h[:, :, i:i + H2R, j:j + W2W].sum(axis=(2, 3),
                                                    dtype=np.float64)
            for b in range(B):
                for k, c in enumerate(live1[b]):
                    q2v[b, c * 9 + i * 3 + j] = s[b, k]
    qh2 = _qhash(q2v, a2n, b2n)
    mask2 = (fh2[None, :] == qh2[:, None])              # [B, 5] bool
    live2 = [np.where(mask2[b])[0] if len(live1[b]) else np.empty(0, np.int64)
             for b in range(B)]
    n2s = max(1, max((len(v) for v in live2), default=1))

    out_full = np.broadcast_to(
        bln[None, None, None, :], (B, 5, H2R, 10)).astype(np.float32).copy()

    work = [b for b in range(B) if len(live1[b]) and len(live2[b])]
    if not work:
        return out_full

    # ---- geometry ----
    UW = min(40, 128 // n1s)       # h1 slab rows (K2 = n1s*UW <= 128 etc.)
    ST = UW - 2
    TPI = -(-H2R // ST)            # tiles per image
    IPC = -(-len(work) // N_CORES)  # images per core (padded)
    K1 = 3 * (UW + 2)
    M1 = n1s * UW
    F2 = n2s * ST

    key = (TPI, IPC, UW, n1s, n2s)
    if key not in _CACHE:
        _CACHE.clear()
        _CACHE[key] = _build_nc(TPI, IPC, UW, n1s, n2s)
    nc = _CACHE[key]

    # ---- host prep: per-core inputs ----
    x16 = x.astype(np.float16)
    wltf = _build_wlt(Wln).astype(np.float16)
    TPC = TPI * IPC
    in_maps = []
    core_imgs = []
    for c in range(N_CORES):
        imgs = work[c * IPC:(c + 1) * IPC]
        core_imgs.append(imgs)
        xs = np.zeros((TPC, K1, WX), np.float16)
        l1c = np.zeros((IPC, K1, 3 * M1), np.float16)
        w2c = np.zeros((IPC, K2loc := n1s * UW, 3 * F2), np.float16)
        for il, b in enumerate(imgs):
            W2g = np.zeros((n2s, n1s, 3, 3), np.float32)
            for ko, co in enumerate(live2[b]):
                for ki, ci in enumerate(live1[b]):
                    W2g[ko, ki] = W2n[co, ci]
            l1c[il] = _build_l1(W1g[b], n1s, UW).astype(np.float16)
            w2c[il] = _build_w2(W2g, n1s, n2s, UW, ST).astype(np.float16)
            for t in range(TPI):
                r0 = ST * t
                nrow = min(UW + 2, H - r0)
                sl = x16[b, :, r0:r0 + nrow, :]           # [3, nrow, 1004]
                dst = xs[il * TPI + t].reshape(3, UW + 2, WX)
                dst[:, :nrow, :] = sl
        boot = np.concatenate([xs[0], l1c[0]], axis=1)
        in_maps.append({"xs": xs, "boot": boot, "l1": l1c, "w2": w2c,
                        "wlt": wltf})

    res = run_bass_kernel_spmd(nc, in_maps, core_ids=list(range(N_CORES)),
                               **kw)
    global LAST_RES
    LAST_RES = res

    # ---- host: scatter ----
    for c in range(N_CORES):
        lin = res.results[c]["out"]                     # [IPC, F2, TPI*10]
        for il, b in enumerate(core_imgs[c]):
            for t in range(TPI):
                r0 = ST * t
                nr = min(ST, H2R - r0)
                blk = lin[il][:, t * 10:(t + 1) * 10].astype(
                    np.float32).reshape(n2s, ST, 10)
                for ko, co in enumerate(live2[b]):
                    out_full[b, co, r0:r0 + nr, :] = blk[ko, :nr, :] \
                        + bln[None, :]
    return out_full
